# revision 18
# baseline (speedup 1.0000x reference)
"""TRN2 Bass kernel for nn_BNN3L (GLIFR recurrent net, T=1000, B=256, H=512).

Strategy (time-parallel SPMD over 8 cores, no collectives):
  - A time chunk can be computed from a zero initial state after a short
    warmup. The slowest state mode decays ~0.5/step (v), so 8 warmup steps
    suffice: chunked-vs-exact error 3.6e-6 l2 in fp64 (validated), far below
    bf16 noise (~2.4e-3). Core 0 owns t in [0,132) exactly (zero init is the
    true initial state); core c>=1 runs 132 iterations on x[124c : 124c+132]
    and owns the last 124. 132 + 7*124 = 1000.

Math refactor (sigma = sigmoid(v/50); constants folded on host):
  The after-spike currents are EMAs of sigma (u' = 0.85u - sigma,
  w' = -0.5w - sigma) with steady-state gains -20/3 and -2/3; they are
  approximated instantaneously and folded as a diagonal into the recurrent
  weights (validated MORE accurate than dropping w alone: 3.0e-6 vs 6.5e-6):
    Wr = (10c*W_rec).T - (20/3 + 2/3)*c*I
    psum = x_t @ (0.5c*W_in).T + sigma @ Wr + [rank-1 c*(0.5*b_in + I0)]
    v' = 0.99*v*(1 - sigma) + psum (+ c*I0 scalar bias fast path)
    out_t = sigma' @ (20*W_out).T + b_out   (b_out added on host)
where c = DT*K_M*R_HID.

Engine balance per half-step (2 batch halves of 128 pipeline across engines):
  PE : 16 rec + 4 in matmuls into psum, then 4 out matmuls (all FD=128)
  DVE: r = v*s99 (early, off-chain), v' = (psum + bias) + r as ONE
       scalar_tensor_tensor reading PSUM directly (no evict!), s99' ts
  ACT: Sigmoid, out-proj evict
The out-proj for step i is emitted after iteration i+1's psum block so the
psum closes as early as possible and PE never head-of-line blocks on the
current step's sigmoid. The serial cycle per half is then just
  sigma -> 20 matmuls -> STT v' -> sigma.
"""
import os
import sys
import numpy as np

for _p in ("/opt/trn_rl_repo", "/root/.axon_site/_ro/trn_rl_repo"):
    if os.path.isdir(_p) and _p not in sys.path:
        sys.path.insert(0, _p)

import ml_dtypes

BF = ml_dtypes.bfloat16

T, B, N_IN, H, O = 1000, 256, 128, 512, 128
NCORES = 8
NITER = 132          # iterations per core
OWN1 = 124           # owned steps per core for cores 1..7 (warmup = 8)
C = float(np.float32(0.05 * 0.2 * (0.1 + 1.0 / H)))
I0 = 700.0

_CACHE = {}


def _build(rank1_const: bool):
    """Build the Bass program. rank1_const: add per-h constant via K=1 matmuls
    (general b_in); otherwise fold the uniform c*I0 into the evict bias."""
    import concourse.bass as bass
    import concourse.mybir as mybir
    from concourse.tile import TileContext
    from concourse.mybir import AluOpType as alu

    F = mybir.ActivationFunctionType
    bf = mybir.dt.bfloat16
    f32 = mybir.dt.float32

    nc = bass.Bass()
    x_d = nc.dram_tensor("x", [NITER, N_IN, B], bf, kind="ExternalInput")
    wrec_d = nc.dram_tensor("wrec", [H, H], bf, kind="ExternalInput")   # [h_in, h_out] = (10c*W_rec).T - (22/3)c*I
    win_d = nc.dram_tensor("win", [N_IN, H], bf, kind="ExternalInput")  # (0.5c*W_in).T
    wout_d = nc.dram_tensor("wout", [H, O], bf, kind="ExternalInput")   # (20*W_out).T
    cvec_d = nc.dram_tensor("cvec", [1, H], bf, kind="ExternalInput")   # c*(0.5*b_in + I0)
    out_d = nc.dram_tensor("out", [NITER, O, B], f32, kind="ExternalOutput")

    XB = 8  # x/out DMA block (steps per transfer)

    with TileContext(nc) as tc:
        with tc.tile_pool(name="const", bufs=1) as cpool, \
             tc.tile_pool(name="state", bufs=1) as spool, \
             tc.tile_pool(name="sig", bufs=4) as sigpool, \
             tc.tile_pool(name="xin", bufs=3) as xpool, \
             tc.tile_pool(name="tmp", bufs=6) as tpool, \
             tc.tile_pool(name="outsb", bufs=3) as opool, \
             tc.tile_pool(name="py", bufs=4, space="PSUM") as pypool, \
             tc.tile_pool(name="po", bufs=4, space="PSUM") as popool:

            # --- constants / weights (resident) ---
            wrec_sb = cpool.tile([128, 4, H], bf)
            nc.sync.dma_start(
                out=wrec_sb[:], in_=wrec_d[:].rearrange("(k p) m -> p k m", p=128))
            win_sb = cpool.tile([128, H], bf)
            nc.sync.dma_start(out=win_sb[:], in_=win_d[:])
            wout_sb = cpool.tile([128, 4, O], bf)
            nc.sync.dma_start(
                out=wout_sb[:], in_=wout_d[:].rearrange("(k p) o -> p k o", p=128))
            if rank1_const:
                cvec_sb = cpool.tile([1, H], bf)
                nc.sync.dma_start(out=cvec_sb[:], in_=cvec_d[:])
                ones_sb = cpool.tile([1, 128], bf)
                nc.vector.memset(ones_sb[:], 1.0)
                yc_bias = 0.0
            else:
                yc_bias = C * I0

            # --- persistent per-half states ---
            v = [spool.tile([128, 512], bf, tag=f"v{h}", name=f"v{h}") for h in (0, 1)]
            sig_p = [sigpool.tile([128, 512], bf, tag=f"sig{h}", name=f"sig{h}") for h in (0, 1)]
            s99_p = [sigpool.tile([128, 512], bf, tag=f"s99{h}", name=f"s99{h}") for h in (0, 1)]
            for h in (0, 1):  # vector memsets: ~0.4us each vs ~2us on gpsimd
                nc.vector.memset(v[h][:], 0.0)
                nc.vector.memset(sig_p[h][:], 0.0)   # s_{-1} = 0 (matches reference)
                nc.vector.memset(s99_p[h][:], 0.99)  # 0.99*(1 - sigma)

            x_blk = None
            out_blk = None
            out_w = 0
            for i in range(NITER + 1):
                j = i - 1  # step whose out-proj/evict is emitted this pass
                if i < NITER:
                    ib = i % XB
                    if ib == 0:
                        xw = min(XB, NITER - i)
                        x_blk = xpool.tile([128, xw, B], bf, name="x_blk")
                        nc.sync.dma_start(
                            out=x_blk[:],
                            in_=x_d[i:i + xw].rearrange("t p b -> p t b"))
                    x_t = x_blk[:, ib, :]
                if j >= 0 and j % XB == 0:
                    out_w = min(XB, NITER - j)
                    out_blk = opool.tile([128, out_w, B], f32, name="out_blk")
                sig_j = [sig_p[0], sig_p[1]]  # sigma produced in iteration j
                for h in (0, 1):
                    bs = slice(h * 128, h * 128 + 128)
                    if i < NITER:
                        sp, s99 = sig_p[h], s99_p[h]
                        # ---- DVE (early, off critical path): r = v * s99 ----
                        r = tpool.tile([128, 512], bf, tag="r")
                        nc.vector.tensor_tensor(r[:], v[h][:], s99[:], alu.mult)
                        # ---- PE: psum[h_lo, (h_hi, b)] accumulation ----
                        psum = pypool.tile([128, 512], mybir.dt.float32, tag="py")
                        for m in range(4):
                            ms = slice(m * 128, m * 128 + 128)
                            # in-proj FIRST: it has no sigma dependency, so the
                            # PE array keeps streaming through the sigma-wait
                            # window and the rec matmuls resume back-to-back
                            # (a queue-head wait on a drained array costs the
                            # full ~219ns isolated-MM latency otherwise)
                            nc.tensor.matmul(psum[:, ms], win_sb[:, ms], x_t[:, bs],
                                             start=True, stop=False)
                            if rank1_const:
                                nc.tensor.matmul(psum[:, ms], cvec_sb[:, ms],
                                                 ones_sb[:], start=False, stop=False)
                            for k in range(4):
                                ks = slice(k * 128, k * 128 + 128)
                                # single stop on the very last psum write:
                                # program order implies earlier m-blocks are
                                # done, and one sem-inc tail beats four
                                nc.tensor.matmul(psum[:, ms], wrec_sb[:, k, ms],
                                                 sp[:, ks], start=False,
                                                 stop=(m == 3 and k == 3))
                        # ---- DVE: v' = (psum + bias) + r, straight from PSUM
                        nc.vector.scalar_tensor_tensor(
                            v[h][:], psum[:], yc_bias, r[:], alu.add, alu.add)
                        # ---- ACT: next sigma; DVE: s99 = 0.99 - 0.99*sigma ----
                        sig_n = sigpool.tile([128, 512], bf, tag=f"sig{h}", name=f"sig{h}")
                        nc.scalar.activation(sig_n[:], v[h][:], F.Sigmoid, scale=0.02)
                        s99_n = sigpool.tile([128, 512], bf, tag=f"s99{h}", name=f"s99{h}")
                        nc.vector.tensor_scalar(s99_n[:], sig_n[:], -0.99, 0.99,
                                                alu.mult, alu.add)
                    if i < NITER:
                        sig_p[h], s99_p[h] = sig_n, s99_n
                # ---- out-proj for step j (sig_j is sigma produced in
                # iteration j; emitted after BOTH halves' psum blocks so the
                # po matmuls act as PE filler during the sigma waits) ----
                if j >= 0:
                    for h in (0, 1):
                        bs = slice(h * 128, h * 128 + 128)
                        po = popool.tile([128, O], f32, tag="po")
                        for k in range(4):
                            nc.tensor.matmul(po[:], wout_sb[:, k, :],
                                             sig_j[h][:, k * 128:k * 128 + 128],
                                             start=(k == 0), stop=(k == 3))
                        nc.scalar.activation(out_blk[:, j % XB, bs], po[:], F.Copy)
                if j >= 0 and (j % XB == out_w - 1):
                    j0 = j - out_w + 1
                    nc.sync.dma_start(
                        out=out_d[j0:j0 + out_w].rearrange("t o b -> o t b"),
                        in_=out_blk[:])

    return nc


_WAIT_LIMITS = {}  # every non-sequencer instruction gets at most 1 sem wait
_WAIT_SKIP = {"InstEventSemaphore", "InstUnconditionalBranch",
              "InstRegisterMove", "InstISA", "InstHalt", "InstNoOp",
              "InstConditionalBranch"}


def _split_waits(nc):
    """Walrus rejects instructions whose on_wait exceeds the ISA struct's sem
    wait slots (1 for DVE S2S2D2 ops, 2 for matmul/act). Tile occasionally
    emits more (slot-reuse WAR + cross-engine RAW). Move the excess onto a
    standalone EventSemaphore (sequencer-level wait, N-capable) inserted just
    before the instruction on the same engine queue."""
    import concourse.mybir as mybir

    n_split = 0
    for f in nc.m.functions:
        for bb in f.blocks:
            il = bb.instructions
            i = 0
            while i < len(il):
                inst = il[i]
                t = type(inst).__name__
                si = inst.sync_info
                if t in _WAIT_SKIP or si is None or not si.on_wait:
                    i += 1
                    continue
                limit = _WAIT_LIMITS.get(t, 1)
                if len(si.on_wait) > limit:
                    keep = list(si.on_wait[:limit])
                    move = list(si.on_wait[limit:])
                    for wj, wt in enumerate(move):
                        ev = mybir.InstEventSemaphore(
                            name=f"evw_split_{n_split}_{wj}",
                            engine=inst.engine,
                            ins=[], outs=[],
                            sync_info=mybir.SyncInfo(on_wait=[wt], on_update=[]),
                        )
                        il.insert(i, ev)
                        i += 1
                    inst.sync_info = mybir.SyncInfo(
                        on_wait=keep, on_update=list(si.on_update or []))
                    n_split += 1
                    i += 1
                else:
                    i += 1
    return n_split


def _prepare(inputs, W_in, b_in, W_rec, W_out, b_out):
    """Host-side folding + sharding. Returns (rank1_const, in_maps, b_out)."""
    x = np.ascontiguousarray(
        np.asarray(inputs, np.float32).transpose(0, 2, 1)).astype(BF)  # [T, N_IN, B]
    W_in = np.asarray(W_in, np.float32)
    W_rec = np.asarray(W_rec, np.float32)
    W_out = np.asarray(W_out, np.float32)
    b_in = np.asarray(b_in, np.float32)
    b_out = np.asarray(b_out, np.float32)

    win_l = np.ascontiguousarray((np.float32(C * 0.5) * W_in).T).astype(np.float32)
    wrec_l = np.ascontiguousarray((np.float32(C * 10.0) * W_rec).T).astype(np.float32)
    # instantaneous-asc diagonal fold: u,w EMA gains -20/3, -2/3
    wrec_l[np.arange(H), np.arange(H)] -= np.float32(C * (20.0 / 3.0 + 2.0 / 3.0))
    win_l = win_l.astype(BF)
    wrec_l = wrec_l.astype(BF)
    wout_l = np.ascontiguousarray((np.float32(20.0) * W_out).T).astype(BF)
    cvec = (np.float32(C) * (np.float32(0.5) * b_in + np.float32(I0))
            ).reshape(1, H).astype(BF)
    rank1_const = bool(np.any(b_in != 0))

    in_maps = []
    for c in range(NCORES):
        x0 = OWN1 * c
        in_maps.append({
            "x": np.ascontiguousarray(x[x0:x0 + NITER]),
            "wrec": wrec_l, "win": win_l, "wout": wout_l, "cvec": cvec,
        })
    return rank1_const, in_maps, b_out


def _assemble(results, b_out):
    out = np.zeros((T, B, O), np.float32)
    for c in range(NCORES):
        dev = results[c]["out"]  # [NITER, O, B]
        if c == 0:
            out[0:NITER] = dev.transpose(0, 2, 1)
        else:
            t0 = NITER + OWN1 * (c - 1)
            out[t0:t0 + OWN1] = dev[NITER - OWN1:].transpose(0, 2, 1)
    if np.any(b_out != 0):
        out += b_out[None, None, :].astype(np.float32)
    return out


def _install_ntff_shim():
    """The image's antenv package lacks axon_hooks; provide it and register
    the ctypes NTFF hook so trace=True works (profiling only)."""
    import types

    try:
        import antenv.axon_hooks  # noqa: F401
        return
    except ImportError:
        pass
    import antenv

    mod = types.ModuleType("antenv.axon_hooks")
    mod._hook = None
    mod.set_axon_ntff_profile_hook = lambda h: setattr(mod, "_hook", h)
    mod.get_axon_ntff_profile_hook = lambda: mod._hook
    sys.modules["antenv.axon_hooks"] = mod
    antenv.axon_hooks = mod
    try:
        sys.path.insert(0, "/root/.axon_site")
        from trn_agent_boot.trn_boot import _ntff_profile_via_ctypes
        mod._hook = _ntff_profile_via_ctypes("/opt/axon/libaxon_pjrt.so")
    except Exception as e:  # profiling degrades; run still works
        print(f"ntff shim: hook unavailable ({e})")


def kernel(inputs, W_in, b_in, W_rec, W_out, b_out, _trace=False):
    if _trace:
        _install_ntff_shim()
    from concourse.bass_utils import run_bass_kernel_spmd

    rank1_const, in_maps, b_out_np = _prepare(
        inputs, W_in, b_in, W_rec, W_out, b_out)
    key = ("nc", rank1_const)
    if key not in _CACHE:
        nc_new = _build(rank1_const)
        _split_waits(nc_new)
        _CACHE[key] = nc_new
    nc = _CACHE[key]
    res = run_bass_kernel_spmd(nc, in_maps, core_ids=list(range(NCORES)),
                               trace=_trace)
    out = _assemble(res.results, b_out_np)
    if _trace:
        kernel.last_exec_time_ns = res.exec_time_ns
    return out


# revision 20
# speedup vs baseline: 1.1245x; 1.1245x over previous
"""TRN2 Bass kernel for nn_BNN3L (GLIFR recurrent net, T=1000, B=256, H=512).

Strategy (time-parallel SPMD over 8 cores, no collectives):
  - A time chunk can be computed from a zero initial state after a short
    warmup. The slowest state mode decays ~0.5/step (v), so 8 warmup steps
    suffice: chunked-vs-exact error 3.6e-6 l2 in fp64 (validated), far below
    bf16 noise (~2.4e-3). Core 0 owns t in [0,132) exactly (zero init is the
    true initial state); core c>=1 runs 132 iterations on x[124c : 124c+132]
    and owns the last 124. 132 + 7*124 = 1000.

Math refactor (sigma = sigmoid(v/50); constants folded on host):
  The after-spike currents are EMAs of sigma (u' = 0.85u - sigma,
  w' = -0.5w - sigma) with steady-state gains -20/3 and -2/3; they are
  approximated instantaneously and folded as a diagonal into the recurrent
  weights (validated MORE accurate than dropping w alone: 3.0e-6 vs 6.5e-6):
    Wr = (10c*W_rec).T - (20/3 + 2/3)*c*I
    psum = x_t @ (0.5c*W_in).T + sigma @ Wr + [rank-1 c*(0.5*b_in + I0)]
    v' = 0.99*v*(1 - sigma) + psum (+ c*I0 scalar bias fast path)
    out_t = sigma' @ (20*W_out).T + b_out   (b_out added on host)
where c = DT*K_M*R_HID.

Engine balance per half-step (2 batch halves of 128 pipeline across engines):
  PE : 16 rec + 4 in matmuls into psum, then 4 out matmuls (all FD=128)
  DVE: r = v*s99 (early, off-chain), v' = (psum + bias) + r as ONE
       scalar_tensor_tensor reading PSUM directly (no evict!), s99' ts
  ACT: Sigmoid, out-proj evict
The out-proj for step i is emitted after iteration i+1's psum block so the
psum closes as early as possible and PE never head-of-line blocks on the
current step's sigmoid. The serial cycle per half is then just
  sigma -> 20 matmuls -> STT v' -> sigma.
"""
import os
import sys
import numpy as np

for _p in ("/opt/trn_rl_repo", "/root/.axon_site/_ro/trn_rl_repo"):
    if os.path.isdir(_p) and _p not in sys.path:
        sys.path.insert(0, _p)

import ml_dtypes

BF = ml_dtypes.bfloat16

T, B, N_IN, H, O = 1000, 256, 128, 512, 128
NCORES = 8
NITER = 132          # iterations per core
OWN1 = 124           # owned steps per core for cores 1..7 (warmup = 8)
C = float(np.float32(0.05 * 0.2 * (0.1 + 1.0 / H)))
I0 = 700.0

_CACHE = {}


def _build(rank1_const: bool):
    """Build the Bass program. rank1_const: add per-h constant via K=1 matmuls
    (general b_in); otherwise fold the uniform c*I0 into the evict bias."""
    import concourse.bass as bass
    import concourse.mybir as mybir
    from concourse.tile import TileContext
    from concourse.mybir import AluOpType as alu

    F = mybir.ActivationFunctionType
    bf = mybir.dt.bfloat16
    f32 = mybir.dt.float32

    nc = bass.Bass()
    x_d = nc.dram_tensor("x", [NITER, N_IN, B], bf, kind="ExternalInput")
    wrec_d = nc.dram_tensor("wrec", [H, H], bf, kind="ExternalInput")   # [h_in, h_out] = (10c*W_rec).T - (22/3)c*I
    win_d = nc.dram_tensor("win", [N_IN, H], bf, kind="ExternalInput")  # (0.5c*W_in).T
    wout_d = nc.dram_tensor("wout", [H, O], bf, kind="ExternalInput")   # (20*W_out).T
    cvec_d = nc.dram_tensor("cvec", [1, H], bf, kind="ExternalInput")   # c*(0.5*b_in + I0)
    out_d = nc.dram_tensor("out", [NITER, O, B], f32, kind="ExternalOutput")

    XB = 8  # x/out DMA block (steps per transfer)

    with TileContext(nc) as tc:
        with tc.tile_pool(name="const", bufs=1) as cpool, \
             tc.tile_pool(name="state", bufs=1) as spool, \
             tc.tile_pool(name="sig", bufs=4) as sigpool, \
             tc.tile_pool(name="xin", bufs=3) as xpool, \
             tc.tile_pool(name="tmp", bufs=6) as tpool, \
             tc.tile_pool(name="outsb", bufs=3) as opool, \
             tc.tile_pool(name="py", bufs=4, space="PSUM") as pypool:

            # --- constants / weights (resident) ---
            wrec_sb = cpool.tile([128, 4, H], bf)
            nc.sync.dma_start(
                out=wrec_sb[:], in_=wrec_d[:].rearrange("(k p) m -> p k m", p=128))
            win_sb = cpool.tile([128, H], bf)
            nc.sync.dma_start(out=win_sb[:], in_=win_d[:])
            wout_sb = cpool.tile([128, 4, O], bf)
            nc.sync.dma_start(
                out=wout_sb[:], in_=wout_d[:].rearrange("(k p) o -> p k o", p=128))
            if rank1_const:
                cvec_sb = cpool.tile([1, H], bf)
                nc.sync.dma_start(out=cvec_sb[:], in_=cvec_d[:])
                ones_sb = cpool.tile([1, 128], bf)
                nc.vector.memset(ones_sb[:], 1.0)
                yc_bias = 0.0
            else:
                yc_bias = C * I0

            # --- persistent per-half states ---
            v = [spool.tile([128, 512], bf, tag=f"v{h}", name=f"v{h}") for h in (0, 1)]
            sig_p = [sigpool.tile([128, 512], bf, tag=f"sig{h}", name=f"sig{h}") for h in (0, 1)]
            s99_p = [sigpool.tile([128, 512], bf, tag=f"s99{h}", name=f"s99{h}") for h in (0, 1)]
            for h in (0, 1):  # vector memsets: ~0.4us each vs ~2us on gpsimd
                nc.vector.memset(v[h][:], 0.0)
                nc.vector.memset(sig_p[h][:], 0.0)   # s_{-1} = 0 (matches reference)
                nc.vector.memset(s99_p[h][:], 0.99)  # 0.99*(1 - sigma)

            x_blk = None
            psum_t = [None, None]
            out_blk = None
            out_w = 0
            for i in range(NITER + 1):
                j = i - 1  # step whose out-proj/evict is emitted this pass
                if i < NITER:
                    ib = i % XB
                    if ib == 0:
                        xw = min(XB, NITER - i)
                        x_blk = xpool.tile([128, xw, B], bf, name="x_blk")
                        nc.sync.dma_start(
                            out=x_blk[:],
                            in_=x_d[i:i + xw].rearrange("t p b -> p t b"))
                    x_t = x_blk[:, ib, :]
                if j >= 0 and j % XB == 0:
                    out_w = min(XB, NITER - j)
                    out_blk = opool.tile([128, out_w, B], f32, name="out_blk")
                sig_j = [sig_p[0], sig_p[1]]  # sigma produced in iteration j
                for h in (0, 1):
                    bs = slice(h * 128, h * 128 + 128)
                    if i < NITER:
                        sp, s99 = sig_p[h], s99_p[h]
                        # ---- DVE (early, off critical path): r = v * s99 ----
                        r = tpool.tile([128, 512], bf, tag="r")
                        nc.vector.tensor_tensor(r[:], v[h][:], s99[:], alu.mult)
                        # ---- PE: psum[h_lo, (h_hi, b)] accumulation.
                        # The tile also carries the po region [512:640] for
                        # step j's out-proj: ONE accumulation-group stop per
                        # half-step (readers spin-waiting on a stop matmul
                        # extend its drain by ~170ns; one beats two) ----
                        psum = pypool.tile([128, 640], mybir.dt.float32, tag="py")
                        psum_t[h] = psum
                        for m in range(4):
                            ms = slice(m * 128, m * 128 + 128)
                            # in-proj FIRST: it has no sigma dependency, so the
                            # PE array keeps streaming through the sigma-wait
                            # window and the rec matmuls resume back-to-back
                            # (a queue-head wait on a drained array costs the
                            # full ~219ns isolated-MM latency otherwise)
                            nc.tensor.matmul(psum[:, ms], win_sb[:, ms], x_t[:, bs],
                                             start=True, stop=False)
                            if rank1_const:
                                nc.tensor.matmul(psum[:, ms], cvec_sb[:, ms],
                                                 ones_sb[:], start=False, stop=False)
                            for k in range(4):
                                ks = slice(k * 128, k * 128 + 128)
                                # single stop on the very last psum write:
                                # program order implies earlier m-blocks are
                                # done, and one sem-inc tail beats four
                                nc.tensor.matmul(psum[:, ms], wrec_sb[:, k, ms],
                                                 sp[:, ks], start=False, stop=False)
                        # ---- DVE: v' = (psum + bias) + r, straight from PSUM
                        nc.vector.scalar_tensor_tensor(
                            v[h][:], psum[:, :512], yc_bias, r[:], alu.add, alu.add)
                        # ---- ACT: next sigma; DVE: s99 = 0.99 - 0.99*sigma ----
                        sig_n = sigpool.tile([128, 512], bf, tag=f"sig{h}", name=f"sig{h}")
                        nc.scalar.activation(sig_n[:], v[h][:], F.Sigmoid, scale=0.02)
                        s99_n = sigpool.tile([128, 512], bf, tag=f"s99{h}", name=f"s99{h}")
                        nc.vector.tensor_scalar(s99_n[:], sig_n[:], -0.99, 0.99,
                                                alu.mult, alu.add)
                    if i < NITER:
                        sig_p[h], s99_p[h] = sig_n, s99_n
                # ---- out-proj for step j (sig_j is sigma produced in
                # iteration j; emitted after BOTH halves' psum blocks so the
                # po matmuls act as PE filler during the sigma waits) ----
                if j >= 0:
                    for h in (0, 1):
                        bs = slice(h * 128, h * 128 + 128)
                        if i >= NITER:  # final pass: po-only tile
                            psum_t[h] = pypool.tile([128, 640], mybir.dt.float32,
                                                    tag="py", name="py_fin")
                        po = psum_t[h][:, 512:640]
                        for k in range(4):
                            nc.tensor.matmul(po, wout_sb[:, k, :],
                                             sig_j[h][:, k * 128:k * 128 + 128],
                                             start=(k == 0), stop=(k == 3))
                        nc.scalar.activation(out_blk[:, j % XB, bs], po, F.Copy)
                if j >= 0 and (j % XB == out_w - 1):
                    j0 = j - out_w + 1
                    nc.sync.dma_start(
                        out=out_d[j0:j0 + out_w].rearrange("t o b -> o t b"),
                        in_=out_blk[:])

    return nc


_WAIT_LIMITS = {}  # every non-sequencer instruction gets at most 1 sem wait
_WAIT_SKIP = {"InstEventSemaphore", "InstUnconditionalBranch",
              "InstRegisterMove", "InstISA", "InstHalt", "InstNoOp",
              "InstConditionalBranch"}


def _split_waits(nc):
    """Walrus rejects instructions whose on_wait exceeds the ISA struct's sem
    wait slots (1 for DVE S2S2D2 ops, 2 for matmul/act). Tile occasionally
    emits more (slot-reuse WAR + cross-engine RAW). Move the excess onto a
    standalone EventSemaphore (sequencer-level wait, N-capable) inserted just
    before the instruction on the same engine queue."""
    import concourse.mybir as mybir

    n_split = 0
    for f in nc.m.functions:
        for bb in f.blocks:
            il = bb.instructions
            i = 0
            while i < len(il):
                inst = il[i]
                t = type(inst).__name__
                si = inst.sync_info
                if t in _WAIT_SKIP or si is None or not si.on_wait:
                    i += 1
                    continue
                limit = _WAIT_LIMITS.get(t, 1)
                if len(si.on_wait) > limit:
                    keep = list(si.on_wait[:limit])
                    move = list(si.on_wait[limit:])
                    for wj, wt in enumerate(move):
                        ev = mybir.InstEventSemaphore(
                            name=f"evw_split_{n_split}_{wj}",
                            engine=inst.engine,
                            ins=[], outs=[],
                            sync_info=mybir.SyncInfo(on_wait=[wt], on_update=[]),
                        )
                        il.insert(i, ev)
                        i += 1
                    inst.sync_info = mybir.SyncInfo(
                        on_wait=keep, on_update=list(si.on_update or []))
                    n_split += 1
                    i += 1
                else:
                    i += 1
    return n_split


def _prepare(inputs, W_in, b_in, W_rec, W_out, b_out):
    """Host-side folding + sharding. Returns (rank1_const, in_maps, b_out)."""
    x = np.ascontiguousarray(
        np.asarray(inputs, np.float32).transpose(0, 2, 1)).astype(BF)  # [T, N_IN, B]
    W_in = np.asarray(W_in, np.float32)
    W_rec = np.asarray(W_rec, np.float32)
    W_out = np.asarray(W_out, np.float32)
    b_in = np.asarray(b_in, np.float32)
    b_out = np.asarray(b_out, np.float32)

    win_l = np.ascontiguousarray((np.float32(C * 0.5) * W_in).T).astype(np.float32)
    wrec_l = np.ascontiguousarray((np.float32(C * 10.0) * W_rec).T).astype(np.float32)
    # instantaneous-asc diagonal fold: u,w EMA gains -20/3, -2/3
    wrec_l[np.arange(H), np.arange(H)] -= np.float32(C * (20.0 / 3.0 + 2.0 / 3.0))
    win_l = win_l.astype(BF)
    wrec_l = wrec_l.astype(BF)
    wout_l = np.ascontiguousarray((np.float32(20.0) * W_out).T).astype(BF)
    cvec = (np.float32(C) * (np.float32(0.5) * b_in + np.float32(I0))
            ).reshape(1, H).astype(BF)
    rank1_const = bool(np.any(b_in != 0))

    in_maps = []
    for c in range(NCORES):
        x0 = OWN1 * c
        in_maps.append({
            "x": np.ascontiguousarray(x[x0:x0 + NITER]),
            "wrec": wrec_l, "win": win_l, "wout": wout_l, "cvec": cvec,
        })
    return rank1_const, in_maps, b_out


def _assemble(results, b_out):
    out = np.zeros((T, B, O), np.float32)
    for c in range(NCORES):
        dev = results[c]["out"]  # [NITER, O, B]
        if c == 0:
            out[0:NITER] = dev.transpose(0, 2, 1)
        else:
            t0 = NITER + OWN1 * (c - 1)
            out[t0:t0 + OWN1] = dev[NITER - OWN1:].transpose(0, 2, 1)
    if np.any(b_out != 0):
        out += b_out[None, None, :].astype(np.float32)
    return out


def _install_ntff_shim():
    """The image's antenv package lacks axon_hooks; provide it and register
    the ctypes NTFF hook so trace=True works (profiling only)."""
    import types

    try:
        import antenv.axon_hooks  # noqa: F401
        return
    except ImportError:
        pass
    import antenv

    mod = types.ModuleType("antenv.axon_hooks")
    mod._hook = None
    mod.set_axon_ntff_profile_hook = lambda h: setattr(mod, "_hook", h)
    mod.get_axon_ntff_profile_hook = lambda: mod._hook
    sys.modules["antenv.axon_hooks"] = mod
    antenv.axon_hooks = mod
    try:
        sys.path.insert(0, "/root/.axon_site")
        from trn_agent_boot.trn_boot import _ntff_profile_via_ctypes
        mod._hook = _ntff_profile_via_ctypes("/opt/axon/libaxon_pjrt.so")
    except Exception as e:  # profiling degrades; run still works
        print(f"ntff shim: hook unavailable ({e})")


def kernel(inputs, W_in, b_in, W_rec, W_out, b_out, _trace=False):
    if _trace:
        _install_ntff_shim()
    from concourse.bass_utils import run_bass_kernel_spmd

    rank1_const, in_maps, b_out_np = _prepare(
        inputs, W_in, b_in, W_rec, W_out, b_out)
    key = ("nc", rank1_const)
    if key not in _CACHE:
        nc_new = _build(rank1_const)
        _split_waits(nc_new)
        _CACHE[key] = nc_new
    nc = _CACHE[key]
    res = run_bass_kernel_spmd(nc, in_maps, core_ids=list(range(NCORES)),
                               trace=_trace)
    out = _assemble(res.results, b_out_np)
    if _trace:
        kernel.last_exec_time_ns = res.exec_time_ns
    return out
